# revision 9
# baseline (speedup 1.0000x reference)
"""Causal multi-head attention (B=2, S=2048, D=1024, H=16) on 8 TRN2 NeuronCores.

Sharding: sequence-parallel. Cores 0-3 handle batch 0, cores 4-7 batch 1.
Within a batch group, the core with local index l owns the mirrored pair of
256-row chunks (A = rows [256l, 256l+256), B = rows [256(7-l), 256(8-l))),
which equalizes causal attention work across cores. A uniform 24-job
structure per head serves all cores from one SPMD program; per-core
host-computed masks select valid/diagonal/invalid kv blocks.

v2 pipeline (vs v1): per-pair interleaved QKV projection + AllGather so the
collective stream starts early; scores for 2 jobs x 2 heads accumulate in a
2-bank PSUM tile so exp (ACT) and mask-mult (DVE) each run once per 2 jobs
on [128,1024]; PV matmuls use the attention weights as the PE-stationary
operand, so the context lands in [q, c] PSUM layout where the softmax
denominator is per-partition: normalization is a local reciprocal +
tensor_scalar multiply (no DRAM broadcast round-trip), keeping the tensor
queue free of long stalls (PE p-state stays high).

Matmuls run in bf16 with fp32 PSUM accumulation (~3e-3 max rel error).
"""

import numpy as np

B, S, D = 2, 2048, 1024
H = 16
HD = 64
NCORES = 8
CHUNK = 256          # rows per chunk; 2 chunks per core
SLOC = 2 * CHUNK     # rows per core
NPAIR = H // 2       # head pairs
NJOB = 24            # uniform job count per head pair: 16 B-phase + 8 A-phase
NFUSE = NJOB // 2    # job pairs fused for exp/mask
KT_P = 128 * SLOC    # kT elems per pair block
V_P = SLOC * 130     # v(+ones) elems per pair block
PAIRSZ = KT_P + V_P

_CACHE = {}


def _build_nc():
    import ml_dtypes
    import concourse.bass as bass
    import concourse.bacc as bacc
    import concourse.mybir as mybir
    import concourse.tile as tile

    f32 = mybir.dt.float32
    bf16 = mybir.dt.bfloat16
    MULT = mybir.AluOpType.mult
    ADD = mybir.AluOpType.add
    EXP = mybir.ActivationFunctionType.Exp

    nc = bacc.Bacc(num_devices=NCORES)

    x_in = nc.dram_tensor("x_local", [SLOC, D], bf16, kind="ExternalInput")
    wqkp_in = nc.dram_tensor("w_qk_p", [128, 16, 8, 128], bf16, kind="ExternalInput")
    wv_in = nc.dram_tensor("w_v_p", [128, NPAIR, 8, 128], bf16, kind="ExternalInput")
    bqk_in = nc.dram_tensor("b_qk_t", [128, 16], f32, kind="ExternalInput")
    bv_in = nc.dram_tensor("b_v_bc", [128, D], f32, kind="ExternalInput")
    wout_in = nc.dram_tensor("w_out", [D, D], bf16, kind="ExternalInput")
    masks_in = nc.dram_tensor("masks2", [128, NFUSE, 1024], bf16, kind="ExternalInput")
    y_out = nc.dram_tensor("y", [SLOC, D], f32, kind="ExternalOutput")

    cc_in = nc.dram_tensor("cc_in", [NPAIR * PAIRSZ], bf16)
    cc_outs = [nc.dram_tensor(f"cc_out_{p}", [4, PAIRSZ], bf16) for p in range(NPAIR)]
    groups = [[0, 1, 2, 3], [4, 5, 6, 7]]

    ident_h = nc.inline_tensor(np.eye(128).astype(ml_dtypes.bfloat16), name="ident_c")
    zeros_h = nc.inline_tensor(np.zeros((1, 512), ml_dtypes.bfloat16), name="zeros_c")

    def kt_view(p, r):   # [128 c, SLOC s] of rank r's pair block
        return cc_outs[p][r, 0:KT_P].rearrange("(c s) -> c s", s=SLOC)

    def v_view(p, r):    # [SLOC s, 130] of rank r's pair block
        return cc_outs[p][r, KT_P:].rearrange("(s c) -> s c", c=130)

    def chunk_owner(c):  # chunk -> (owner rank, row offset in that rank's 512)
        return (c, 0) if c < 4 else (7 - c, CHUNK)

    with tile.TileContext(nc) as tc:
        with tc.tile_pool(name="const", bufs=1) as cpool:
            it = cpool.tile([128, 128], bf16)
            nc.sync.dma_start(out=it[:], in_=ident_h[:])
            zt = cpool.tile([1, 512], bf16)
            nc.sync.dma_start(out=zt[:], in_=zeros_h[:])
            bqk = cpool.tile([128, 16], f32)
            nc.sync.dma_start(out=bqk[:], in_=bqk_in[:])
            bv = cpool.tile([128, D], f32)
            nc.sync.dma_start(out=bv[:], in_=bv_in[:])
            xT = cpool.tile([128, 8, SLOC], bf16)
            qT = cpool.tile([128, 8, SLOC], bf16)
            ctxT = cpool.tile([128, 8, SLOC], bf16)
            masks = cpool.tile([128, NFUSE, 1024], bf16)
            wo = cpool.tile([128, 8, D], bf16)

            # ---- Phase 1: load x, transpose to xT ----
            with tc.tile_pool(name="ph1", bufs=3) as pool, \
                 tc.tile_pool(name="ph1p", bufs=4, space="PSUM") as psp:
                for sb in range(4):
                    xl = pool.tile([128, D], bf16, tag="xl")
                    nc.sync.dma_start(out=xl[:], in_=x_in[sb * 128:(sb + 1) * 128, :])
                    for db in range(8):
                        pst = psp.tile([128, 128], bf16, tag="tr")
                        nc.tensor.transpose(pst[:], xl[:, db * 128:(db + 1) * 128], it[:])
                        nc.vector.tensor_copy(out=xT[:, db, sb * 128:(sb + 1) * 128], in_=pst[:])

            # ---- Phase 2: per-pair QKV projection + AllGather ----
            with tc.tile_pool(name="ph2w", bufs=3) as wpool, \
                 tc.tile_pool(name="ph2wv", bufs=2) as wvpool, \
                 tc.tile_pool(name="ph2", bufs=3) as pool, \
                 tc.tile_pool(name="ph2p", bufs=2, space="PSUM") as psp:

                def qk_block(cb, out_sb, nm):
                    wp = wpool.tile([128, 8, 128], bf16, tag="wp", name=f"wp_{nm}")
                    nc.sync.dma_start(out=wp[:], in_=wqkp_in[:, cb, :, :])
                    ps = psp.tile([128, SLOC], f32, tag="ps", name=f"psqk_{nm}")
                    for db in range(8):
                        nc.tensor.matmul(ps[:], wp[:, db, :], xT[:, db, :],
                                         start=(db == 0), stop=(db == 7))
                    nc.vector.tensor_scalar_add(out_sb, ps[:], bqk[:, cb:cb + 1])

                for p in range(NPAIR):
                    # k block for pair p -> cc_in kT region
                    kt = pool.tile([128, SLOC], bf16, tag="kt", name=f"kt_{p}")
                    qk_block(8 + p, kt[:], f"k{p}")
                    nc.sync.dma_start(
                        out=cc_in[p * PAIRSZ: p * PAIRSZ + KT_P]
                            .rearrange("(c s) -> c s", s=SLOC),
                        in_=kt[:])
                    # v block for pair p (128 cols + ones) -> cc_in v region
                    wv = wvpool.tile([128, 8, 128], bf16, tag="wv", name=f"wv_{p}")
                    nc.sync.dma_start(out=wv[:], in_=wv_in[:, p, :, :])
                    for sb in range(4):
                        ps = psp.tile([128, 128], f32, tag="ps", name=f"psv_{p}_{sb}")
                        for db in range(8):
                            nc.tensor.matmul(ps[:], xT[:, db, sb * 128:(sb + 1) * 128],
                                             wv[:, db, :], start=(db == 0), stop=(db == 7))
                        vt = pool.tile([128, 130], bf16, tag="vt", name=f"vt_{p}_{sb}")
                        nc.vector.tensor_tensor(
                            out=vt.rearrange("p (h c) -> p h c", h=2)[:, :, 0:64],
                            in0=ps.rearrange("p (h c) -> p h c", h=2),
                            in1=bv[:, p * 128:(p + 1) * 128]
                                .rearrange("p (h c) -> p h c", h=2),
                            op=ADD)
                        nc.vector.memset(vt[:, 64:65], 1.0)
                        nc.vector.memset(vt[:, 129:130], 1.0)
                        nc.sync.dma_start(
                            out=cc_in[p * PAIRSZ + KT_P:(p + 1) * PAIRSZ]
                                .rearrange("(s c) -> s c", c=130)[sb * 128:(sb + 1) * 128, :],
                            in_=vt[:])
                    # AllGather pair p (kT + v together)
                    nc.gpsimd.collective_compute(
                        "AllGather", mybir.AluOpType.bypass, replica_groups=groups,
                        ins=[cc_in[p * PAIRSZ:(p + 1) * PAIRSZ]],
                        outs=[cc_outs[p][:]],
                    )
                    # q block for pair p -> qT resident (overlaps the AG)
                    qk_block(p, qT[:, p, :], f"q{p}")

                # bulk loads needed later; issued after all AG-critical DMAs
                nc.sync.dma_start(out=masks[:], in_=masks_in[:])
                for cb in range(8):
                    nc.sync.dma_start(out=wo[:, cb, :],
                                      in_=wout_in[cb * 128:(cb + 1) * 128, :])

            # ---- Phase 3: attention, one head-pair at a time ----
            with tc.tile_pool(name="kv", bufs=2) as kvpool, \
                 tc.tile_pool(name="at", bufs=3) as atpool, \
                 tc.tile_pool(name="cq", bufs=3) as cqpool, \
                 tc.tile_pool(name="misc", bufs=2) as mpool, \
                 tc.tile_pool(name="ps_s", bufs=4, space="PSUM") as spool, \
                 tc.tile_pool(name="ps_c", bufs=1, space="PSUM") as ctxpool:
                for p in range(NPAIR):
                    # kT_pair [128, 16, 128]: global kv-block order
                    ktp = kvpool.tile([128, 16, 128], bf16, tag="ktp", name=f"ktp_{p}")
                    for c in range(8):
                        r, off = chunk_owner(c)
                        nc.sync.dma_start(
                            out=ktp[:, 2 * c:2 * c + 2, :],
                            in_=kt_view(p, r)[:, off:off + CHUNK]
                                .rearrange("p (b k) -> p b k", b=2),
                        )
                    # v_ext [128, 16, 130]: slots in global kv order (ones baked in)
                    vxt = kvpool.tile([128, 16, 130], bf16, tag="vxt", name=f"vxt_{p}")
                    for c in range(8):
                        r, off = chunk_owner(c)
                        nc.sync.dma_start(
                            out=vxt[:, 2 * c:2 * c + 2, :],
                            in_=v_view(p, r)[off:off + CHUNK, :]
                                .rearrange("(o q) k -> q o k", q=128),
                        )

                    # ctx accumulators, [q, c] layout: [128 q, 4 qtiles, 128]
                    # (only cols 0:65 of each qtile slot used; col 64 = denom)
                    psc = [ctxpool.tile([128, 4, 128], f32, tag=f"ctx{h}",
                                        name=f"ctx_p{p}_{h}")
                           for h in range(2)]
                    for h in range(2):
                        nc.tensor.matmul(
                            psc[h].rearrange("p a b -> p (a b)"),
                            zt[0:1, 0:128], zt[0:1, 0:512],
                            start=True, stop=False, skip_group_check=True)

                    for i in range(NJOB):
                        f, j2 = divmod(i, 2)
                        kv = (15 - i) if i < 16 else (23 - i)
                        choff = CHUNK if i < 16 else 0
                        qtb = 2 if i < 16 else 0
                        last = i == NJOB - 1
                        pss = [spool.tile([128, 256], f32, tag="s",
                                          name=f"s_{p}_{i}_{h}") for h in range(2)]
                        for h in range(2):
                            nc.tensor.matmul(
                                pss[h][:],
                                ktp[h * 64:(h + 1) * 64, kv, :],
                                qT[h * 64:(h + 1) * 64, p, choff:choff + CHUNK],
                                start=True, stop=True, tile_position=(h * 64, 0),
                            )
                        ats = atpool.tile([128, 2, 256], bf16, tag="at",
                                          name=f"at_{p}_{i}")
                        for h in range(2):
                            nc.scalar.activation(ats[:, h, :], pss[h][:], EXP,
                                                 scale=0.125)
                        nc.vector.tensor_tensor(
                            out=ats.rearrange("p a b -> p (a b)"),
                            in0=ats.rearrange("p a b -> p (a b)"),
                            in1=masks[:, f, j2 * 512:(j2 + 1) * 512], op=MULT)
                        for h in range(2):
                            for qh in range(2):
                                nc.tensor.matmul(
                                    psc[h][:, qtb + qh, 0:65],
                                    ats[:, h, qh * 128:(qh + 1) * 128],
                                    vxt[:, kv, h * 65:h * 65 + 65],
                                    start=False, stop=(last and qh == 1),
                                    skip_group_check=True,
                                )

                    # normalize + transpose to ctxT[:, p, :]
                    rc = [mpool.tile([128, 4, 1], f32, tag=f"rc{h}", name=f"rc_{p}_{h}")
                          for h in range(2)]
                    for h in range(2):
                        with nc.allow_low_precision(reason="softmax denom"):
                            nc.vector.reciprocal(rc[h][:], psc[h][:, :, 64:65])
                    for qt in range(4):
                        cq = cqpool.tile([128, 128], bf16, tag="cq", name=f"cq_{p}_{qt}")
                        for h in range(2):
                            nc.vector.tensor_scalar_mul(
                                cq[:, h * 64:(h + 1) * 64],
                                psc[h][:, qt, 0:64],
                                rc[h][:, qt, 0:1])
                        pst = spool.tile([128, 128], bf16, tag="s", name=f"tr_{p}_{qt}")
                        nc.tensor.transpose(pst[:], cq[:], it[:])
                        nc.vector.tensor_copy(
                            out=ctxT[:, p, qt * 128:(qt + 1) * 128], in_=pst[:])

            # ---- Phase 4: output projection ----
            with tc.tile_pool(name="ph4", bufs=3) as pool, \
                 tc.tile_pool(name="ph4p", bufs=2, space="PSUM") as psp:
                for sb in range(4):
                    for nb in range(2):
                        ps = psp.tile([128, 512], f32, tag="y", name=f"psy_{sb}_{nb}")
                        for cb in range(8):
                            nc.tensor.matmul(ps[:], ctxT[:, cb, sb * 128:(sb + 1) * 128],
                                             wo[:, cb, nb * 512:(nb + 1) * 512],
                                             start=(cb == 0), stop=(cb == 7))
                        yt = pool.tile([128, 512], f32, tag="yt", name=f"yt_{sb}_{nb}")
                        nc.vector.tensor_copy(out=yt[:], in_=ps[:])
                        nc.sync.dma_start(
                            out=y_out[sb * 128:(sb + 1) * 128, nb * 512:(nb + 1) * 512],
                            in_=yt[:])

    nc.finalize()
    return nc


def _host_inputs(x, W_qkv, b_qkv, W_out):
    import ml_dtypes

    x = np.asarray(x, ml_dtypes.bfloat16)
    W_qkv = np.asarray(W_qkv, np.float32)
    b_qkv = np.asarray(b_qkv, np.float32)
    W_out = np.ascontiguousarray(np.asarray(W_out, ml_dtypes.bfloat16))

    # q/k panels: [p, cb, db, c] = W_qkv[db*128+p, cb*128+c] for cb in 0..15
    wqk = W_qkv[:, :2 * D].reshape(8, 128, 16, 128)          # [db, p, cb, c]
    wqk_p = np.ascontiguousarray(wqk.transpose(1, 2, 0, 3).astype(ml_dtypes.bfloat16))
    # v panels: [p, pair, db, c] = W_qkv[db*128+p, 2D + pair*128 + c]
    wv = W_qkv[:, 2 * D:].reshape(8, 128, NPAIR, 128)        # [db, p, pair, c]
    wv_p = np.ascontiguousarray(wv.transpose(1, 2, 0, 3).astype(ml_dtypes.bfloat16))

    bqk_t = np.ascontiguousarray(b_qkv[:2 * D].reshape(16, 128).T)  # [128, 16]
    bv_bc = np.ascontiguousarray(np.broadcast_to(b_qkv[2 * D:], (128, D)))

    in_maps = []
    for c in range(NCORES):
        b, l = divmod(c, 4)
        cA, cB = l, 7 - l
        x_local = np.ascontiguousarray(
            np.concatenate([x[b, cA * CHUNK:(cA + 1) * CHUNK],
                            x[b, cB * CHUNK:(cB + 1) * CHUNK]], axis=0))
        # fused masks: [128, NFUSE, 2 jobs, 2 heads, 256] -> [128, NFUSE, 1024]
        m2 = np.zeros((128, NFUSE, 2, 2, CHUNK), np.float32)
        pp = np.arange(128)[:, None]
        ff = np.arange(CHUNK)[None, :]
        for i in range(NJOB):
            if i < 16:
                kvb, r0 = 15 - i, cB * CHUNK
            else:
                kvb, r0 = 23 - i, cA * CHUNK
            mm = (128 * kvb + pp <= r0 + ff).astype(np.float32)
            f, j2 = divmod(i, 2)
            m2[:, f, j2, 0] = mm
            m2[:, f, j2, 1] = mm
        in_maps.append({
            "x_local": x_local,
            "w_qk_p": wqk_p,
            "w_v_p": wv_p,
            "b_qk_t": bqk_t,
            "b_v_bc": bv_bc,
            "w_out": W_out,
            "masks2": m2.reshape(128, NFUSE, 1024).astype(ml_dtypes.bfloat16),
        })
    return in_maps


def _run(in_maps, trace=False):
    from concourse.bass_utils import run_bass_kernel_spmd

    if "nc" not in _CACHE:
        _CACHE["nc"] = _build_nc()
    return run_bass_kernel_spmd(_CACHE["nc"], in_maps, core_ids=list(range(NCORES)),
                                trace=trace)


def kernel(x, W_qkv, b_qkv, W_out):
    in_maps = _host_inputs(x, W_qkv, b_qkv, W_out)
    res = _run(in_maps)
    out = np.empty((B, S, D), np.float32)
    for c in range(NCORES):
        b, l = divmod(c, 4)
        y = res.results[c]["y"]
        out[b, l * CHUNK:(l + 1) * CHUNK] = y[0:CHUNK]
        out[b, (7 - l) * CHUNK:(8 - l) * CHUNK] = y[CHUNK:2 * CHUNK]
    return out


# revision 13
# speedup vs baseline: 1.0504x; 1.0504x over previous
"""Causal multi-head attention (B=2, S=2048, D=1024, H=16) on 8 TRN2 NeuronCores.

Sharding: sequence-parallel. Cores 0-3 handle batch 0, cores 4-7 batch 1.
Within a batch group, the core with local index l owns the mirrored pair of
256-row chunks (A = rows [256l, 256l+256), B = rows [256(7-l), 256(8-l))),
which equalizes causal attention work across cores. A uniform 24-job
structure per head serves all cores from one SPMD program; per-core
host-computed masks select valid/diagonal/invalid kv blocks.

v2 pipeline (vs v1): per-pair interleaved QKV projection + AllGather so the
collective stream starts early; scores for 2 jobs x 2 heads accumulate in a
2-bank PSUM tile so exp (ACT) and mask-mult (DVE) each run once per 2 jobs
on [128,1024]; PV matmuls use the attention weights as the PE-stationary
operand, so the context lands in [q, c] PSUM layout where the softmax
denominator is per-partition: normalization is a local reciprocal +
tensor_scalar multiply (no DRAM broadcast round-trip), keeping the tensor
queue free of long stalls (PE p-state stays high).

Matmuls run in bf16 with fp32 PSUM accumulation (~3e-3 max rel error).
"""

import numpy as np

B, S, D = 2, 2048, 1024
H = 16
HD = 64
NCORES = 8
CHUNK = 256          # rows per chunk; 2 chunks per core
SLOC = 2 * CHUNK     # rows per core
NPAIR = H // 2       # head pairs
NJOB = 24            # uniform job count per head pair: 16 B-phase + 8 A-phase
NFUSE = NJOB // 2    # job pairs fused for exp/mask
KT_P = 128 * SLOC    # kT elems per pair block
V_P = SLOC * 130     # v(+ones) elems per pair block
PAIRSZ = KT_P + V_P

_CACHE = {}


def _build_nc():
    import ml_dtypes
    import concourse.bass as bass
    import concourse.bacc as bacc
    import concourse.mybir as mybir
    import concourse.tile as tile

    f32 = mybir.dt.float32
    bf16 = mybir.dt.bfloat16
    MULT = mybir.AluOpType.mult
    ADD = mybir.AluOpType.add
    EXP = mybir.ActivationFunctionType.Exp

    nc = bacc.Bacc(num_devices=NCORES)

    x_in = nc.dram_tensor("x_local", [SLOC, D], bf16, kind="ExternalInput")
    wqkp_in = nc.dram_tensor("w_qk_p", [128, 16, 8, 128], bf16, kind="ExternalInput")
    wv_in = nc.dram_tensor("w_v_p", [128, NPAIR, 8, 128], bf16, kind="ExternalInput")
    bqk_in = nc.dram_tensor("b_qk_t", [128, 16], f32, kind="ExternalInput")
    bv_in = nc.dram_tensor("b_v_bc", [128, D], f32, kind="ExternalInput")
    wout_in = nc.dram_tensor("w_out", [D, D], bf16, kind="ExternalInput")
    masks_in = nc.dram_tensor("masks2", [128, NFUSE, 1024], bf16, kind="ExternalInput")
    y_out = nc.dram_tensor("y", [SLOC, D], f32, kind="ExternalOutput")

    cc_in = nc.dram_tensor("cc_in", [NPAIR * PAIRSZ], bf16)
    cc_outs = [nc.dram_tensor(f"cc_out_{p}", [4, PAIRSZ], bf16) for p in range(NPAIR)]
    groups = [[0, 1, 2, 3], [4, 5, 6, 7]]

    ident_h = nc.inline_tensor(np.eye(128).astype(ml_dtypes.bfloat16), name="ident_c")
    zeros_h = nc.inline_tensor(np.zeros((1, 512), ml_dtypes.bfloat16), name="zeros_c")

    def slot(g):
        # ktp/vxt slot for global kv block g: slot = 4*rank + 2*half + b,
        # where half=0 is the owner's A chunk (rows 0:256), half=1 its B chunk.
        c, b = divmod(g, 2)
        return 4 * c + b if c < 4 else 4 * (7 - c) + 2 + b

    with tile.TileContext(nc) as tc:
        with tc.tile_pool(name="const", bufs=1) as cpool:
            it = cpool.tile([128, 128], bf16)
            nc.sync.dma_start(out=it[:], in_=ident_h[:])
            zt = cpool.tile([1, 512], bf16)
            nc.sync.dma_start(out=zt[:], in_=zeros_h[:])
            bqk = cpool.tile([128, 16], f32)
            nc.sync.dma_start(out=bqk[:], in_=bqk_in[:])
            bv = cpool.tile([128, D], f32)
            nc.sync.dma_start(out=bv[:], in_=bv_in[:])
            xT = cpool.tile([128, 8, SLOC], bf16)
            qT = cpool.tile([128, 8, SLOC], bf16)
            ctxT = cpool.tile([128, 8, SLOC], bf16)
            masks = cpool.tile([128, NFUSE, 1024], bf16)
            wo = cpool.tile([128, 8, D], bf16)

            # bulk loads on the ACT DMA queue (idle until attention starts)
            nc.scalar.dma_start(out=masks[:], in_=masks_in[:])
            for cb in range(8):
                nc.scalar.dma_start(out=wo[:, cb, :],
                                    in_=wout_in[cb * 128:(cb + 1) * 128, :])

            # ---- Phase 1: load x transposed straight into xT (DMA XBAR) ----
            for db in range(8):
                nc.sync.dma_start_transpose(
                    out=xT[:, db, :], in_=x_in[:, db * 128:(db + 1) * 128])

            # ---- Phase 2: per-pair QKV projection + AllGather ----
            with tc.tile_pool(name="ph2w", bufs=3) as wpool, \
                 tc.tile_pool(name="ph2wv", bufs=2) as wvpool, \
                 tc.tile_pool(name="ph2", bufs=3) as pool, \
                 tc.tile_pool(name="ph2p", bufs=2, space="PSUM") as psp:

                def qk_block(cb, out_sb, nm):
                    wp = wpool.tile([128, 8, 128], bf16, tag="wp", name=f"wp_{nm}")
                    nc.sync.dma_start(out=wp[:], in_=wqkp_in[:, cb, :, :])
                    ps = psp.tile([128, SLOC], f32, tag="ps", name=f"psqk_{nm}")
                    for db in range(8):
                        nc.tensor.matmul(ps[:], wp[:, db, :], xT[:, db, :],
                                         start=(db == 0), stop=(db == 7))
                    nc.vector.tensor_scalar_add(out_sb, ps[:], bqk[:, cb:cb + 1])

                for p in range(NPAIR):
                    # k block for pair p -> cc_in kT region
                    kt = pool.tile([128, SLOC], bf16, tag="kt", name=f"kt_{p}")
                    qk_block(8 + p, kt[:], f"k{p}")
                    nc.scalar.dma_start(
                        out=cc_in[p * PAIRSZ: p * PAIRSZ + KT_P]
                            .rearrange("(c s) -> c s", s=SLOC),
                        in_=kt[:])
                    # v block for pair p (128 cols + ones) -> cc_in v region
                    wv = wvpool.tile([128, 8, 128], bf16, tag="wv", name=f"wv_{p}")
                    nc.sync.dma_start(out=wv[:], in_=wv_in[:, p, :, :])
                    for sb in range(4):
                        ps = psp.tile([128, 128], f32, tag="ps", name=f"psv_{p}_{sb}")
                        for db in range(8):
                            nc.tensor.matmul(ps[:], xT[:, db, sb * 128:(sb + 1) * 128],
                                             wv[:, db, :], start=(db == 0), stop=(db == 7))
                        vt = pool.tile([128, 130], bf16, tag="vt", name=f"vt_{p}_{sb}")
                        nc.vector.tensor_tensor(
                            out=vt.rearrange("p (h c) -> p h c", h=2)[:, :, 0:64],
                            in0=ps.rearrange("p (h c) -> p h c", h=2),
                            in1=bv[:, p * 128:(p + 1) * 128]
                                .rearrange("p (h c) -> p h c", h=2),
                            op=ADD)
                        nc.vector.memset(vt[:, 64:65], 1.0)
                        nc.vector.memset(vt[:, 129:130], 1.0)
                        nc.scalar.dma_start(
                            out=cc_in[p * PAIRSZ + KT_P:(p + 1) * PAIRSZ]
                                .rearrange("(s c) -> s c", c=130)[sb * 128:(sb + 1) * 128, :],
                            in_=vt[:])
                    # AllGather pair p (kT + v together)
                    nc.gpsimd.collective_compute(
                        "AllGather", mybir.AluOpType.bypass, replica_groups=groups,
                        ins=[cc_in[p * PAIRSZ:(p + 1) * PAIRSZ]],
                        outs=[cc_outs[p][:]],
                    )
                    # q block for pair p -> qT resident (overlaps the AG)
                    qk_block(p, qT[:, p, :], f"q{p}")

            # ---- Phase 3: attention, one head-pair at a time ----
            with tc.tile_pool(name="kv", bufs=2) as kvpool, \
                 tc.tile_pool(name="at", bufs=3) as atpool, \
                 tc.tile_pool(name="cq", bufs=3) as cqpool, \
                 tc.tile_pool(name="misc", bufs=2) as mpool, \
                 tc.tile_pool(name="ps_s", bufs=4, space="PSUM") as spool, \
                 tc.tile_pool(name="ps_c", bufs=1, space="PSUM") as ctxpool:
                for p in range(NPAIR):
                    # kT_pair [128, 16, 128]: slot layout per slot() above
                    ktp = kvpool.tile([128, 16, 128], bf16, tag="ktp", name=f"ktp_{p}")
                    nc.sync.dma_start(
                        out=ktp.rearrange("c (r t) k -> c r (t k)", r=4),
                        in_=cc_outs[p][:, 0:KT_P]
                            .rearrange("r (c s) -> c r s", c=128),
                    )
                    # v_ext [128, 16, 130]: same slot layout (ones baked in)
                    vxt = kvpool.tile([128, 16, 130], bf16, tag="vxt", name=f"vxt_{p}")
                    vsrc = cc_outs[p][:, KT_P:].rearrange(
                        "r (h b q ck) -> h b q r ck", h=2, b=2, q=128)
                    for hh in range(2):
                        for bb in range(2):
                            nc.sync.dma_start(
                                out=vxt.rearrange("q (r m) ck -> q m r ck",
                                                  m=4)[:, 2 * hh + bb, :, :],
                                in_=vsrc[hh, bb],
                            )

                    # ctx accumulators, [q, c] layout: [128 q, 4 qtiles, 128]
                    # (only cols 0:65 of each qtile slot used; col 64 = denom)
                    psc = [ctxpool.tile([128, 4, 128], f32, tag=f"ctx{h}",
                                        name=f"ctx_p{p}_{h}")
                           for h in range(2)]
                    for h in range(2):
                        nc.tensor.matmul(
                            psc[h].rearrange("p a b -> p (a b)"),
                            zt[0:1, 0:128], zt[0:1, 0:512],
                            start=True, stop=False, skip_group_check=True)

                    for i in range(NJOB):
                        f, j2 = divmod(i, 2)
                        kv = (15 - i) if i < 16 else (23 - i)
                        choff = CHUNK if i < 16 else 0
                        qtb = 2 if i < 16 else 0
                        last = i == NJOB - 1
                        pss = [spool.tile([128, 256], f32, tag="s",
                                          name=f"s_{p}_{i}_{h}") for h in range(2)]
                        for h in range(2):
                            nc.tensor.matmul(
                                pss[h][:],
                                ktp[h * 64:(h + 1) * 64, slot(kv), :],
                                qT[h * 64:(h + 1) * 64, p, choff:choff + CHUNK],
                                start=True, stop=True, tile_position=(h * 64, 0),
                            )
                        ats = atpool.tile([128, 2, 256], bf16, tag="at",
                                          name=f"at_{p}_{i}")
                        for h in range(2):
                            nc.scalar.activation(ats[:, h, :], pss[h][:], EXP,
                                                 scale=0.125)
                        nc.vector.tensor_tensor(
                            out=ats.rearrange("p a b -> p (a b)"),
                            in0=ats.rearrange("p a b -> p (a b)"),
                            in1=masks[:, f, j2 * 512:(j2 + 1) * 512], op=MULT)
                        for h in range(2):
                            for qh in range(2):
                                nc.tensor.matmul(
                                    psc[h][:, qtb + qh, 0:65],
                                    ats[:, h, qh * 128:(qh + 1) * 128],
                                    vxt[:, slot(kv), h * 65:h * 65 + 65],
                                    start=False, stop=(last and qh == 1),
                                    skip_group_check=True,
                                )

                    # normalize + transpose to ctxT[:, p, :]
                    rc = [mpool.tile([128, 4, 1], f32, tag=f"rc{h}", name=f"rc_{p}_{h}")
                          for h in range(2)]
                    for h in range(2):
                        with nc.allow_low_precision(reason="softmax denom"):
                            nc.vector.reciprocal(rc[h][:], psc[h][:, :, 64:65])
                    for qt in range(4):
                        cq = cqpool.tile([128, 128], bf16, tag="cq", name=f"cq_{p}_{qt}")
                        for h in range(2):
                            nc.vector.tensor_scalar_mul(
                                cq[:, h * 64:(h + 1) * 64],
                                psc[h][:, qt, 0:64],
                                rc[h][:, qt, 0:1])
                        pst = spool.tile([128, 128], bf16, tag="s", name=f"tr_{p}_{qt}")
                        nc.tensor.transpose(pst[:], cq[:], it[:])
                        nc.vector.tensor_copy(
                            out=ctxT[:, p, qt * 128:(qt + 1) * 128], in_=pst[:])

            # ---- Phase 4: output projection ----
            with tc.tile_pool(name="ph4", bufs=3) as pool, \
                 tc.tile_pool(name="ph4p", bufs=2, space="PSUM") as psp:
                for sb in range(4):
                    for nb in range(2):
                        ps = psp.tile([128, 512], f32, tag="y", name=f"psy_{sb}_{nb}")
                        for cb in range(8):
                            nc.tensor.matmul(ps[:], ctxT[:, cb, sb * 128:(sb + 1) * 128],
                                             wo[:, cb, nb * 512:(nb + 1) * 512],
                                             start=(cb == 0), stop=(cb == 7))
                        yt = pool.tile([128, 512], f32, tag="yt", name=f"yt_{sb}_{nb}")
                        nc.vector.tensor_copy(out=yt[:], in_=ps[:])
                        nc.sync.dma_start(
                            out=y_out[sb * 128:(sb + 1) * 128, nb * 512:(nb + 1) * 512],
                            in_=yt[:])

    nc.finalize()
    return nc


def _host_inputs(x, W_qkv, b_qkv, W_out):
    import ml_dtypes

    x = np.asarray(x, ml_dtypes.bfloat16)
    W_qkv = np.asarray(W_qkv, np.float32)
    b_qkv = np.asarray(b_qkv, np.float32)
    W_out = np.ascontiguousarray(np.asarray(W_out, ml_dtypes.bfloat16))

    # q/k panels: [p, cb, db, c] = W_qkv[db*128+p, cb*128+c] for cb in 0..15
    wqk = W_qkv[:, :2 * D].reshape(8, 128, 16, 128)          # [db, p, cb, c]
    wqk_p = np.ascontiguousarray(wqk.transpose(1, 2, 0, 3).astype(ml_dtypes.bfloat16))
    # v panels: [p, pair, db, c] = W_qkv[db*128+p, 2D + pair*128 + c]
    wv = W_qkv[:, 2 * D:].reshape(8, 128, NPAIR, 128)        # [db, p, pair, c]
    wv_p = np.ascontiguousarray(wv.transpose(1, 2, 0, 3).astype(ml_dtypes.bfloat16))

    bqk_t = np.ascontiguousarray(b_qkv[:2 * D].reshape(16, 128).T)  # [128, 16]
    bv_bc = np.ascontiguousarray(np.broadcast_to(b_qkv[2 * D:], (128, D)))

    in_maps = []
    for c in range(NCORES):
        b, l = divmod(c, 4)
        cA, cB = l, 7 - l
        x_local = np.ascontiguousarray(
            np.concatenate([x[b, cA * CHUNK:(cA + 1) * CHUNK],
                            x[b, cB * CHUNK:(cB + 1) * CHUNK]], axis=0))
        # fused masks: [128, NFUSE, 2 jobs, 2 heads, 256] -> [128, NFUSE, 1024]
        m2 = np.zeros((128, NFUSE, 2, 2, CHUNK), np.float32)
        pp = np.arange(128)[:, None]
        ff = np.arange(CHUNK)[None, :]
        for i in range(NJOB):
            if i < 16:
                kvb, r0 = 15 - i, cB * CHUNK
            else:
                kvb, r0 = 23 - i, cA * CHUNK
            mm = (128 * kvb + pp <= r0 + ff).astype(np.float32)
            f, j2 = divmod(i, 2)
            m2[:, f, j2, 0] = mm
            m2[:, f, j2, 1] = mm
        in_maps.append({
            "x_local": x_local,
            "w_qk_p": wqk_p,
            "w_v_p": wv_p,
            "b_qk_t": bqk_t,
            "b_v_bc": bv_bc,
            "w_out": W_out,
            "masks2": m2.reshape(128, NFUSE, 1024).astype(ml_dtypes.bfloat16),
        })
    return in_maps


def _run(in_maps, trace=False):
    from concourse.bass_utils import run_bass_kernel_spmd

    if "nc" not in _CACHE:
        _CACHE["nc"] = _build_nc()
    return run_bass_kernel_spmd(_CACHE["nc"], in_maps, core_ids=list(range(NCORES)),
                                trace=trace)


def kernel(x, W_qkv, b_qkv, W_out):
    in_maps = _host_inputs(x, W_qkv, b_qkv, W_out)
    res = _run(in_maps)
    out = np.empty((B, S, D), np.float32)
    for c in range(NCORES):
        b, l = divmod(c, 4)
        y = res.results[c]["y"]
        out[b, l * CHUNK:(l + 1) * CHUNK] = y[0:CHUNK]
        out[b, (7 - l) * CHUNK:(8 - l) * CHUNK] = y[CHUNK:2 * CHUNK]
    return out
